# revision 26
# baseline (speedup 1.0000x reference)
"""Trainium2 Bass kernel for nn_Ag3SRModel (GNN message passing, 4096 atoms).

reference math:
  d_ij pairwise distances, mask = (d>0) & (d<5)
  rbf_k(d) = exp(-(d - k/3)^2 * 4.5), k=0..15
  features[i,k] = sum_j mask * rbf_k(d_ij)
  e = silu(features @ W1 + b1) @ W2 + b2 ; out = sum(e)

Strategy (8 NeuronCores, SPMD):
  The seed-0 positions are spatially clustered, so a kd-tree (median split,
  longest axis) gives 32 tiles of exactly 128 atoms with tight bboxes. For
  each tile the host computes the exact candidate set {j : min_i d_ij < 5}
  (~450-1130 atoms, vs 4096 dense) and packs it into [128 i x 512 j] chunks
  (last chunk padded with a far point P0, >=5A from every real atom so the
  cutoff fold zeroes it). 61 real + 3 dummy chunks = 8 chunks x 8 cores,
  each chunk an independent unit: per-chunk partial features are summed on
  the host.

  Per chunk on device:
    d^2 via augmented matmul (contraction K=5) into a PSUM bank
    cutoff fold on DVE: d'^2 = max(d^2, 144*(d^2>=25))  (masked d' in
      [12,~24] where every rbf underflows to exactly 0)
    d = sqrt(d'^2) (ACT), t = exp(3d) bf16 (ACT)
    anchors k in {0,2,4,6,8,10,12}: one ACT Derivative_Erf pass each:
      a_k = (2/sqrt(pi)) * exp(-(sqrt(4.5) d - sqrt(4.5) c_k)^2)
      with accum_out -> feature column k reduced for free
    k in {1,3,5,7,9,11} + chain 13,14,15: one DVE affine_mul_reduce each:
      out = (prev * alpha_k) * t,  accum_out = sum_j out
      alpha_k = exp(-(2k-1)/2) keeps every tile at true-rbf magnitude
  ACT table thrash is avoided by phase-batching chunks in groups of 4:
  [sqrt x4][exp x4][derf/amr x4] costs 3 table loads per group.

  Host: sum chunk partials, scale by sqrt(pi)/2, subtract diagonal rbf_k(0),
  tiny MLP in f64, total energy.
"""

import math
import sys

sys.path.insert(0, "/opt/trn_rl_repo")

import numpy as np

import concourse.bass as bass
import concourse.tile as tile
from concourse import bacc, mybir
from concourse.bass_utils import run_bass_kernel_spmd

N = 4096
NCORES = 8
P = 128                     # partitions / atoms per kd tile
CW = 512                    # chunk width (j columns)
NRBF = 16
CUTOFF = 5.0
INV2W2 = 4.5                # 1/(2 w^2), w = 1/3
SQ = math.sqrt(INV2W2)
GAUSS_NORM = math.sqrt(math.pi) / 2.0   # undo derivative_erf's 2/sqrt(pi)
# per-chunk anchor sets (k's computed on ACT via Derivative_Erf; the rest
# chain on DVE via affine_mul_reduce from k-1). Tuned so ACT/DVE loads
# balance (anchor ~993ns ACT, chain ~775ns DVE) with the last chunk
# anchor-heavy: it fills ACT's tail idle while DVE drains its backlog.
ANCHORS6 = (0, 2, 4, 6, 8, 10)
ANCHORS7 = (0, 2, 4, 6, 8, 10, 12)
ANCHORS10 = (0, 2, 4, 5, 6, 8, 9, 10, 12, 14)


def _anchor_set(c, K):
    if c == K - 1:
        return ANCHORS10
    return ANCHORS6 if c < 4 else ANCHORS7
PAD_POINT = np.array([7.5, 7.5, 21.0])  # >=5A from box, <=23.6A away
F32 = mybir.dt.float32
BF16 = mybir.dt.bfloat16

_CACHE = {}


def _build(K):
    """Device program: K independent [128 x 512] chunks per core."""
    nc = bacc.Bacc("TRN2", target_bir_lowering=False, debug=False,
                   num_devices=NCORES)

    # per chunk: 128 lhsT cols + 512 rhs cols. 15 rows = split-precision
    # bf16 attrs: lhsT [hi;lo;hi], rhs [hi;hi;lo] so the K=15 contraction
    # yields hi*hi + lo*hi + hi*lo ~ full product to ~5e-3 (one fast bf16
    # matmul instead of a 2-pass fp32 one).
    ab_d = nc.dram_tensor("AB", [15, K * (P + CW)], BF16,
                          kind="ExternalInput").ap()
    feats_d = nc.dram_tensor("feats", [K * P, NRBF], F32,
                             kind="ExternalOutput").ap()

    A = mybir.ActivationFunctionType
    ALU = mybir.AluOpType

    with tile.TileContext(nc) as tc:
        with (
            tc.tile_pool(name="singles", bufs=1) as singles,
            tc.tile_pool(name="dtile", bufs=K + 1) as dpool,
            tc.tile_pool(name="ttile", bufs=K + 1) as tpool,
            tc.tile_pool(name="d2c", bufs=K + 1) as d2cpool,
            tc.tile_pool(name="m144", bufs=2) as mpool,
            tc.tile_pool(name="anch", bufs=8) as apool,
            tc.tile_pool(name="scr", bufs=2) as spool,
            tc.tile_pool(name="fraw", bufs=4) as fpool,
            tc.tile_pool(name="psum_d2", bufs=8, space="PSUM") as psum_d2,
        ):
            ab_sb = singles.tile([15, K * (P + CW)], BF16)
            all_anchor_ks = sorted({k for c in range(K)
                                    for k in _anchor_set(c, K)} - {0})
            biases = {}
            for k in all_anchor_ks:
                b = singles.tile([P, 1], F32, tag=f"bias{k}")
                nc.vector.memset(b, -SQ * (k / 3.0))
                biases[k] = b

            def off(c):
                return c * (P + CW)

            def mm(c):
                # per-chunk input DMA so chunk 0 computes ~immediately
                nc.sync.dma_start(out=ab_sb[:, off(c):off(c + 1)],
                                  in_=ab_d[:, off(c):off(c + 1)])
                ps = psum_d2.tile([P, CW], F32, tag="d2")
                nc.tensor.matmul(ps, ab_sb[:, off(c):off(c) + P],
                                 ab_sb[:, off(c) + P:off(c + 1)],
                                 start=True, stop=True)
                return ps

            def fold(c, ps):
                m = mpool.tile([P, CW], F32, tag="m144")
                nc.vector.tensor_scalar(m, ps, CUTOFF * CUTOFF, 144.0,
                                        ALU.is_ge, ALU.mult)
                d2c = d2cpool.tile([P, CW], F32, tag="d2c")
                nc.vector.tensor_tensor(d2c, ps, m, ALU.max)
                return d2c

            def act_core(c, d2c):
                d = dpool.tile([P, CW], F32, tag="d")
                nc.scalar.activation(d, d2c, A.Sqrt)
                return d

            def act_exp(c, d):
                t = tpool.tile([P, CW], BF16, tag="t")
                nc.scalar.activation(t, d, A.Exp, scale=3.0, bias=gate0)
                return t

            def chunk_tail(c, d, t):
                """derf anchors (ACT, free reduce) + amr chain steps (DVE)."""
                anchors = _anchor_set(c, K)
                fraw = fpool.tile([P, NRBF], F32, tag="fraw")
                tiles = {}
                for k in range(NRBF):
                    if k in anchors:
                        ak = apool.tile([P, CW], BF16, tag="anch")
                        nc.scalar.activation(
                            ak, d, A.Derivative_Erf, scale=SQ,
                            accum_out=fraw[:, k:k + 1],
                            bias=gated_biases[k])
                        tiles[k] = ak
                    else:
                        out = spool.tile([P, CW], BF16, tag="scr")
                        nc.vector.affine_mul_reduce(
                            out, fraw[:, k:k + 1], tiles[k - 1], t,
                            scale=math.exp(-(2 * k - 1) / 2.0), bias=0.0)
                        tiles[k] = out
                nc.sync.dma_start(out=feats_d[c * P:(c + 1) * P, :], in_=fraw)

            # flat pipeline: all matmuls up front (8 PSUM banks); DVE folds
            # run during ACT's sqrt phase. The greedy tile scheduler would
            # interleave sqrt/exp/derf and thrash ACT tables, so phase
            # order is FORCED via tiny DVE-produced gate deps: every exp
            # waits on sqrt(K-1) (through its bias AP), every derf on
            # exp(K-1). Exactly 3 ACT table loads.
            psums = [mm(c) for c in range(K)]
            d2cs = [fold(c, psums[c]) for c in range(K)]
            ds = [act_core(c, d2cs[c]) for c in range(K)]
            gate0 = singles.tile([P, 1], F32, tag="gate0")
            nc.vector.tensor_scalar(gate0, ds[K - 1][:, 0:1], 0.0, None,
                                    ALU.mult)
            ts = [act_exp(c, ds[c]) for c in range(K)]
            gateT = singles.tile([P, 1], F32, tag="gateT")
            nc.vector.tensor_scalar(gateT, ts[K - 1][:, 0:1], 0.0, None,
                                    ALU.mult)
            gated_biases = {0: gateT}
            for k in all_anchor_ks:
                gb = singles.tile([P, 1], F32, tag=f"gbias{k}")
                nc.vector.tensor_tensor(gb, biases[k], gateT, ALU.add)
                gated_biases[k] = gb
            for c in range(K):
                chunk_tail(c, ds[c], ts[c])

    nc.compile()
    return nc


def _kdtiles(pos, idx):
    if len(idx) <= P:
        return [idx]
    p = pos[idx]
    ax = int(np.argmax(p.max(0) - p.min(0)))
    o = np.argsort(p[:, ax], kind="stable")
    h = len(idx) // 2
    return _kdtiles(pos, idx[o[:h]]) + _kdtiles(pos, idx[o[h:]])


def _prep(positions):
    """kd tiling + exact candidate sets + chunk packing."""
    pos = positions.astype(np.float64)
    tiles = _kdtiles(pos, np.arange(len(pos)))
    chunks = []          # (tile_atom_idx [128], j_atom_idx [<=512])
    for tidx in tiles:
        p = pos[tidx]
        bd2 = (np.maximum(0.0, np.maximum(p.min(0)[None] - pos,
                                          pos - p.max(0)[None])) ** 2).sum(1)
        cand = np.where(bd2 < CUTOFF * CUTOFF)[0]
        d2 = ((pos[cand][:, None, :] - p[None, :, :]) ** 2).sum(-1)
        cand = cand[(d2 < CUTOFF * CUTOFF).any(1)]
        for s in range(0, len(cand), CW):
            chunks.append((tidx, cand[s:s + CW]))
    K = (len(chunks) + NCORES - 1) // NCORES
    # greedy: distribute chunks round-robin (they are near-uniform cost)
    percore = [[] for _ in range(NCORES)]
    for i, ch in enumerate(chunks):
        percore[i % NCORES].append(ch)
    return K, percore


def _bf16(a):
    """Round f64 -> bf16 values (returned as float64)."""
    u = a.astype(np.float32).view(np.uint32)
    u2 = u + 0x7FFF + ((u >> 16) & 1)   # round-to-nearest-even
    return (u2 & 0xFFFF0000).astype(np.uint32).view(np.float32).astype(np.float64)


def _pack(pos, percore, K):
    """Build per-core AB input arrays (split-precision bf16) + metadata."""
    import ml_dtypes
    pos64 = pos.astype(np.float64)
    in_maps, meta = [], []
    for c in range(NCORES):
        ab = np.zeros((5, K * (P + CW)), dtype=np.float64)
        mm = []
        for s in range(K):
            o = s * (P + CW)
            if s < len(percore[c]):
                tidx, jidx = percore[c][s]
                ti = pos64[tidx]                       # [128, 3]
                tj = pos64[jidx]                       # [<=512, 3]
                nj = len(jidx)
                ab[0:3, o:o + P] = -2.0 * ti.T
                ab[3, o:o + P] = 1.0
                ab[4, o:o + P] = (ti ** 2).sum(1)
                ab[0:3, o + P:o + P + nj] = tj.T
                ab[3, o + P:o + P + nj] = (tj ** 2).sum(1)
                ab[4, o + P:o + P + nj] = 1.0
                if nj < CW:
                    ab[0:3, o + P + nj:o + P + CW] = PAD_POINT[:, None]
                    ab[3, o + P + nj:o + P + CW] = (PAD_POINT ** 2).sum()
                    ab[4, o + P + nj:o + P + CW] = 1.0
                mm.append(tidx)
            else:
                # dummy chunk: real atoms as i (any tile), all-pad j's.
                # Every pad pair is >=5A so it folds to zero contribution;
                # avoids degenerate all-zero d^2 tiles feeding the gates.
                ti = pos64[percore[c][0][0]] if percore[c] else \
                    np.tile(PAD_POINT, (P, 1))
                ab[0:3, o:o + P] = -2.0 * ti.T
                ab[3, o:o + P] = 1.0
                ab[4, o:o + P] = (ti ** 2).sum(1)
                ab[0:3, o + P:o + P + CW] = PAD_POINT[:, None]
                ab[3, o + P:o + P + CW] = (PAD_POINT ** 2).sum()
                ab[4, o + P:o + P + CW] = 1.0
                mm.append(None)
        hi = _bf16(ab)
        lo = _bf16(ab - hi)
        # lhsT cols get [hi; lo; hi], rhs cols get [hi; hi; lo]:
        # contraction = hi*hi + lo*hi + hi*lo (drops only lo*lo ~ 1e-3)
        ab15 = np.zeros((15, K * (P + CW)), dtype=np.float64)
        ab15[0:5] = hi
        for s in range(K):
            o = s * (P + CW)
            ab15[5:10, o:o + P] = lo[:, o:o + P]
            ab15[10:15, o:o + P] = hi[:, o:o + P]
            ab15[5:10, o + P:o + P + CW] = hi[:, o + P:o + P + CW]
            ab15[10:15, o + P:o + P + CW] = lo[:, o + P:o + P + CW]
        in_maps.append({"AB": ab15.astype(ml_dtypes.bfloat16)})
        meta.append(mm)
    return in_maps, meta


def kernel(positions, W1, b1, W2, b2):
    positions = np.asarray(positions, dtype=np.float32)
    W1 = np.asarray(W1, dtype=np.float32)
    b1 = np.asarray(b1, dtype=np.float32)
    W2 = np.asarray(W2, dtype=np.float32)
    b2 = np.asarray(b2, dtype=np.float32)

    K, percore = _prep(positions)
    if ("nc", K) not in _CACHE:
        _CACHE[("nc", K)] = _build(K)
    nc = _CACHE[("nc", K)]
    _CACHE["last"] = (nc, K, percore)

    in_maps, meta = _pack(positions, percore, K)
    res = run_bass_kernel_spmd(nc, in_maps, list(range(NCORES)))

    feats = np.zeros((N, NRBF), dtype=np.float64)
    for c in range(NCORES):
        fr = res.results[c]["feats"].astype(np.float64)   # [K*128, 16]
        for s, tidx in enumerate(meta[c]):
            if tidx is not None:
                feats[tidx] += fr[s * P:(s + 1) * P]
    feats *= GAUSS_NORM

    ks = np.arange(NRBF, dtype=np.float64)
    ek = np.exp(-0.5 * ks * ks)             # diagonal rbf_k(0)
    f = (feats - ek).astype(np.float32)

    z = (f @ W1 + b1).astype(np.float64)
    h = z * 0.5 * (1.0 + np.tanh(0.5 * z))  # silu, overflow-safe
    e = h @ W2.reshape(-1, 1) + b2.reshape(1, -1)
    return np.float32(e.sum())
